# revision 1
# baseline (speedup 1.0000x reference)
"""Trainium2 Bass kernel for nn_DecoderRNN greedy-decode LSTM.

Strategy (8 NeuronCores, SPMD):
  - Vocab-parallel: each core holds a [H, V/8] slice of the fc weight and
    computes its [B, V/8] logits slice each decode step.
  - LSTM recurrence (B=64, H=512) is replicated on every core.
  - Greedy-argmax feedback needs a global argmax over V=32000: each core
    computes its local (max, argmax, sum-of-exp) and a tiny [64, 4] AllGather
    per step combines them; every core then gathers the winning embedding row
    from its own replica of the embedding table via indirect DMA.
  - Softmax normalization: p = exp(l) / sum(exp(l)) without max-subtraction
    (logits are tiny: |l| < ~0.5, so exp cannot overflow); the global sum is
    assembled from the 8 per-core partial sums carried by the same AllGather.
  - Sigmoid is computed as sig(x) = (tanh(x/2)+1)/2 so every activation
    (tanh/exp/copy) lives in the single "exp_and_others" ACT table set.
    To avoid an extra 0.5x scaling op, the kernel tracks h2 = 2*h and
    c2 = 2*c; W_hh and W_fc are pre-scaled by 0.5 on the host.
"""

import sys

sys.path.insert(0, "/opt/trn_rl_repo")

import numpy as np
from contextlib import ExitStack

import concourse.bass as bass
import concourse.bacc as bacc
import concourse.mybir as mybir
from concourse.tile import TileContext
from concourse.masks import make_identity
from concourse.bass_utils import run_bass_kernel_spmd

B, T, E, H, V = 64, 32, 256, 512, 32000
NCORES = 8
VC = V // NCORES          # 4000 vocab columns per core
NCH = 8                   # fc column chunks per core
CW = VC // NCH            # 500 columns per chunk

F32 = mybir.dt.float32
I32 = mybir.dt.int32
U32 = mybir.dt.uint32
AF = mybir.ActivationFunctionType
OP = mybir.AluOpType
AX = mybir.AxisListType

_CACHE = {}


import os
NSTEPS = int(os.environ.get("KSTEPS", str(T)))


def _build():
    nc = bacc.Bacc("TRN2", target_bir_lowering=False, debug=False,
                   num_devices=NCORES)

    featT = nc.dram_tensor("featT", [E, B], F32, kind="ExternalInput")
    wg = nc.dram_tensor("wg", [6 * 128, 4 * H], F32, kind="ExternalInput")
    wgb = nc.dram_tensor("wgb", [1, 4 * H], F32, kind="ExternalInput")
    wf = nc.dram_tensor("wf", [H, VC], F32, kind="ExternalInput")
    wfb = nc.dram_tensor("wfb", [1, VC], F32, kind="ExternalInput")
    emb = nc.dram_tensor("emb", [V, E], F32, kind="ExternalInput")
    outp = nc.dram_tensor("outp", [B, T - 1, VC], F32, kind="ExternalOutput")

    with TileContext(nc) as tc, ExitStack() as ctx:
        const = ctx.enter_context(tc.tile_pool(name="const", bufs=1))
        sb1 = ctx.enter_context(tc.tile_pool(name="sb1", bufs=1))
        sb2 = ctx.enter_context(tc.tile_pool(name="sb2", bufs=2))
        xb = ctx.enter_context(tc.tile_pool(name="xb", bufs=2))
        dram = ctx.enter_context(tc.tile_pool(name="dram", bufs=2, space="DRAM"))
        gp = ctx.enter_context(tc.tile_pool(name="gp", bufs=1, space="PSUM"))
        fcp = ctx.enter_context(tc.tile_pool(name="fcp", bufs=2, space="PSUM"))
        tpp = ctx.enter_context(tc.tile_pool(name="tpp", bufs=2, space="PSUM"))

        # ---- constants ----
        W6 = const.tile([128, 6, 4 * H], F32)
        nc.sync.dma_start(out=W6, in_=wg[:, :].rearrange("(c p) n -> p c n", p=128))
        Wgb = const.tile([1, 4 * H], F32)
        nc.sync.dma_start(out=Wgb, in_=wgb[:, :])
        Wf4 = const.tile([128, 4, VC], F32)
        nc.sync.dma_start(out=Wf4, in_=wf[:, :].rearrange("(c p) n -> p c n", p=128))
        Wfb = const.tile([1, VC], F32)
        nc.sync.dma_start(out=Wfb, in_=wfb[:, :])
        featT_s = const.tile([128, 2, B], F32)
        nc.sync.dma_start(out=featT_s, in_=featT[:, :].rearrange("(c p) b -> p c b", p=128))
        ones1 = const.tile([1, B], F32)
        nc.vector.memset(ones1, 1.0)
        ident = const.tile([B, B], F32)
        make_identity(nc, ident)
        K8i = const.tile([B, 8], I32)
        nc.gpsimd.iota(K8i, pattern=[[1, 8]], base=0, channel_multiplier=0)
        K8f = const.tile([B, 8], F32)
        nc.vector.tensor_copy(K8f, K8i)
        zeros512 = const.tile([B, H], F32)
        nc.vector.memset(zeros512, 0.0)

        xT_cur = featT_s
        h2T_cur = None
        c2_cur = zeros512

        STAGE = int(os.environ.get("K_STAGE", "99"))
        for j in range(NSTEPS):
            use_h = j >= 2
            # ---- gates: G = x @ W_ih.T + h @ (0.5*W_hh).T + (b_ih+b_hh) ----
            G = gp.tile([B, 4 * H], F32, name=f"G_{j}", tag="G")
            lhs = [xT_cur[:, 0, :], xT_cur[:, 1, :]]
            rhs = [W6[:, 0], W6[:, 1]]
            if use_h:
                lhs += [h2T_cur[:, c, :] for c in range(4)]
                rhs += [W6[:, c + 2] for c in range(4)]
            lhs.append(ones1[:, :])
            rhs.append(Wgb)
            for n in range(4):
                sl = slice(n * 512, (n + 1) * 512)
                for i, (lh, rh) in enumerate(zip(lhs, rhs)):
                    nc.tensor.matmul(G[:, sl], lh, rh[:, sl],
                                     start=(i == 0), stop=(i == len(lhs) - 1))

            if STAGE < 1:
                continue
            # ---- gate activations: t = tanh(gate/2) (i,f,o), tanh(g) ----
            tg4 = sb1.tile([B, 4 * H], F32, name=f"tg4_{j}", tag="tg4")
            for (st, en, sc) in ((0, H, 0.5), (H, 2 * H, 0.5),
                                 (2 * H, 3 * H, 1.0), (3 * H, 4 * H, 0.5)):
                nc.scalar.activation(tg4[:, st:en], G[:, st:en], AF.Tanh, scale=sc)
            ti = tg4[:, 0:H]
            tf_ = tg4[:, H:2 * H]
            tgg = tg4[:, 2 * H:3 * H]
            to_ = tg4[:, 3 * H:4 * H]

            if STAGE < 2:
                continue
            # ---- cell: c2' = (tf+1)*c2/2 + (ti+1)*tg ;  h2 = (to+1)*tanh(c2'/2)
            ab = sb1.tile([B, 2 * H], F32, name=f"ab_{j}", tag="ab")
            nc.vector.scalar_tensor_tensor(out=ab[:, 0:H], in0=tf_, scalar=1.0,
                                           in1=c2_cur, op0=OP.add, op1=OP.mult)
            nc.vector.scalar_tensor_tensor(out=ab[:, H:2 * H], in0=ti, scalar=1.0,
                                           in1=tgg, op0=OP.add, op1=OP.mult)
            c2n = sb2.tile([B, H], F32, name=f"c2_{j}", tag="c2")
            nc.vector.scalar_tensor_tensor(out=c2n, in0=ab[:, 0:H], scalar=0.5,
                                           in1=ab[:, H:2 * H], op0=OP.mult, op1=OP.add)
            tcn = sb1.tile([B, H], F32, name=f"tc_{j}", tag="tc")
            nc.scalar.activation(tcn, c2n, AF.Tanh, scale=0.5)
            h2 = sb1.tile([B, H], F32, name=f"h2_{j}", tag="h2")
            nc.vector.scalar_tensor_tensor(out=h2, in0=to_, scalar=1.0,
                                           in1=tcn, op0=OP.add, op1=OP.mult)

            if STAGE < 3:
                continue
            # ---- transpose h2 -> h2T [128, 4, B] for use as matmul lhsT ----
            h2T = xb.tile([128, 4, B], F32, name=f"h2T_{j}", tag="h2T")
            for c in range(4):
                tp = tpp.tile([128, B], F32, name=f"tph_{j}_{c}", tag="tp")
                nc.tensor.transpose(tp, h2[:, c * 128:(c + 1) * 128], ident)
                nc.vector.tensor_copy(h2T[:, c, :], tp)

            if STAGE < 4:
                continue
            # ---- fc: logits chunks; fused chunk max + exp(+accum) ----
            expv = sb2.tile([B, VC], F32, name=f"expv_{j}", tag="expv")
            cmax = sb2.tile([B, NCH, 8], F32, name=f"cmax_{j}", tag="cmax")
            esum = sb2.tile([B, NCH], F32, name=f"esum_{j}", tag="esum")
            for n in range(NCH):
                sl = slice(n * CW, (n + 1) * CW)
                L = fcp.tile([B, CW], F32, name=f"L_{j}_{n}", tag="L")
                for c in range(4):
                    nc.tensor.matmul(L, h2T[:, c, :], Wf4[:, c, sl],
                                     start=(c == 0), stop=False)
                nc.tensor.matmul(L, ones1[:, :], Wfb[:, sl], start=False, stop=True)
                nc.vector.max(cmax[:, n, :], L)
                nc.scalar.activation(expv[:, sl], L, AF.Exp,
                                     accum_out=esum[:, n:n + 1])
            m8 = sb2.tile([B, 8], F32, name=f"m8_{j}", tag="m8")
            nc.vector.max(m8, cmax)

            if STAGE < 5:
                continue
            # ---- pack (m, local_idx, local_sum) and AllGather ----
            pk = sb2.tile([B, 4], F32, name=f"pk_{j}", tag="pk")
            nc.vector.tensor_copy(pk[:, 0:1], m8[:, 0:1])
            if j <= T - 2:
                em = sb2.tile([B, 1], F32, name=f"em_{j}", tag="em")
                nc.scalar.activation(em, m8[:, 0:1], AF.Exp)
                idx8 = sb2.tile([B, 8], U32, name=f"idx8_{j}", tag="idx8")
                if os.environ.get("K_NOMAXIDX"):
                    nc.vector.memset(idx8, 0)
                else:
                    nc.vector.max_index(idx8, em.to_broadcast([B, 8]), expv)
                nc.vector.tensor_copy(pk[:, 1:2], idx8[:, 0:1])
            else:
                nc.vector.memset(pk[:, 1:2], 0.0)
            nc.vector.reduce_sum(pk[:, 2:3], esum, axis=AX.X)
            nc.vector.memset(pk[:, 3:4], 0.0)

            cc_in = dram.tile([B, 4], F32, name=f"ccin_{j}", tag="ccin")
            cc_out = dram.tile([NCORES * B, 4], F32, name=f"ccout_{j}", tag="ccout")
            nc.sync.dma_start(out=cc_in[:], in_=pk)
            if os.environ.get("K_NOCC"):
                for _kk in range(NCORES):
                    nc.sync.dma_start(out=cc_out[_kk * B:(_kk + 1) * B, :], in_=pk)
            else:
                nc.gpsimd.collective_compute(
                    "AllGather", OP.bypass,
                    replica_groups=[list(range(NCORES))],
                    ins=[cc_in.opt()], outs=[cc_out.opt()],
                )
            A = sb2.tile([B, NCORES, 4], F32, name=f"A_{j}", tag="A")
            nc.sync.dma_start(out=A, in_=cc_out[:].rearrange("(k b) c -> b k c", k=NCORES))

            if STAGE < 6:
                continue
            # ---- global sum -> 1/s (needed for output steps j>=1) ----
            if j >= 1:
                st_ = sb2.tile([B, 1], F32, name=f"st_{j}", tag="st")
                nc.vector.reduce_sum(st_, A[:, :, 2], axis=AX.X)
                rs = sb2.tile([B, 1], F32, name=f"rs_{j}", tag="rs")
                nc.vector.reciprocal(rs, st_)

            if STAGE < 7:
                continue
            WSUB = int(os.environ.get("K_WSUB", "99"))
            # ---- winner core + embedding gather (all steps but the last) ----
            if j <= T - 2:
                g8 = sb2.tile([B, 8], F32, name=f"g8_{j}", tag="g8")
                nc.vector.max(g8, A[:, :, 0])
                if WSUB < 2:
                    continue
                k8 = sb2.tile([B, 8], U32, name=f"k8_{j}", tag="k8")
                nc.vector.max_index(k8, g8, A[:, :, 0])
                if WSUB < 3:
                    continue
                kf = sb2.tile([B, 1], F32, name=f"kf_{j}", tag="kf")
                nc.vector.tensor_copy(kf, k8[:, 0:1])
                msk = sb2.tile([B, 8], F32, name=f"msk_{j}", tag="msk")
                nc.vector.tensor_scalar(msk, K8f, kf, None, OP.is_equal)
                if WSUB < 4:
                    continue
                ttrj = sb2.tile([B, 8], F32, name=f"ttrj_{j}", tag="ttrj")
                idxsel = sb2.tile([B, 1], F32, name=f"idxsel_{j}", tag="idxsel")
                nc.vector.tensor_tensor(out=ttrj, in0=msk, in1=A[:, :, 1], op=OP.mult)
                nc.vector.reduce_sum(idxsel, ttrj, axis=AX.X)
                gidxf = sb2.tile([B, 1], F32, name=f"gidxf_{j}", tag="gidxf")
                nc.vector.scalar_tensor_tensor(out=gidxf, in0=kf, scalar=float(VC),
                                               in1=idxsel, op0=OP.mult, op1=OP.add)
                gidx = sb2.tile([B, 1], I32, name=f"gidx_{j}", tag="gidx")
                nc.vector.tensor_copy(gidx, gidxf)
                if WSUB < 5:
                    continue
                xn = sb2.tile([B, E], F32, name=f"xn_{j}", tag="xn")
                if os.environ.get("K_NOIND"):
                    nc.sync.dma_start(out=xn, in_=emb[0:B, :])
                else:
                    nc.gpsimd.indirect_dma_start(
                        out=xn, out_offset=None, in_=emb[:, :],
                        in_offset=bass.IndirectOffsetOnAxis(ap=gidx[:, :1], axis=0))
                xT = xb.tile([128, 2, B], F32, name=f"xT_{j}", tag="xT")
                for c in range(2):
                    tp = tpp.tile([128, B], F32, name=f"tpx_{j}_{c}", tag="tp")
                    nc.tensor.transpose(tp, xn[:, c * 128:(c + 1) * 128], ident)
                    nc.vector.tensor_copy(xT[:, c, :], tp)
                xT_cur = xT

            if STAGE < 8:
                continue
            # ---- normalize p = expv * (1/s) and store ----
            if j >= 1:
                nc.vector.tensor_scalar(expv, expv, rs, None, OP.mult)
                nc.sync.dma_start(out=outp[:, j - 1, :], in_=expv)

            h2T_cur = h2T
            c2_cur = c2n if j >= 1 else zeros512

    nc.compile()
    return nc


def _prep_inputs(features, captions, embed_table, W_ih, W_hh, b_ih, b_hh,
                 W_fc, b_fc):
    features = np.asarray(features, dtype=np.float32)
    embed_table = np.ascontiguousarray(np.asarray(embed_table, dtype=np.float32))
    W_ih = np.asarray(W_ih, dtype=np.float32)
    W_hh = np.asarray(W_hh, dtype=np.float32)
    b_ih = np.asarray(b_ih, dtype=np.float32)
    b_hh = np.asarray(b_hh, dtype=np.float32)
    W_fc = np.asarray(W_fc, dtype=np.float32)
    b_fc = np.asarray(b_fc, dtype=np.float32)

    featT = np.ascontiguousarray(features.T)                       # [E, B]
    wg = np.ascontiguousarray(
        np.concatenate([W_ih.T, 0.5 * W_hh.T], axis=0))            # [768, 2048]
    wgb = np.ascontiguousarray((b_ih + b_hh)[None, :])             # [1, 2048]
    common = {"featT": featT, "wg": wg, "wgb": wgb, "emb": embed_table}
    in_maps = []
    for k in range(NCORES):
        v0 = k * VC
        wfk = np.ascontiguousarray(0.5 * W_fc[v0:v0 + VC].T)       # [H, VC]
        wfbk = np.ascontiguousarray(b_fc[v0:v0 + VC][None, :])     # [1, VC]
        in_maps.append(dict(common, wf=wfk, wfb=wfbk))
    return in_maps


def kernel(**inputs):
    if "nc" not in _CACHE:
        _CACHE["nc"] = _build()
    nc = _CACHE["nc"]
    in_maps = _prep_inputs(**inputs)
    res = run_bass_kernel_spmd(nc, in_maps, core_ids=list(range(NCORES)))
    out = np.zeros((B, T, V), dtype=np.float32)
    for k in range(NCORES):
        nts = max(NSTEPS - 1, 0)
        out[:, :nts, k * VC:(k + 1) * VC] = res.results[k]["outp"][:, :nts]
    return out


if __name__ == "__main__":
    rng = np.random.default_rng(0)
    ins = {
        "features": rng.normal(size=(B, E)).astype(np.float32),
        "captions": rng.integers(0, V, size=(B, T)).astype(np.int64),
        "embed_table": (rng.normal(size=(V, E)) * 0.02).astype(np.float32),
        "W_ih": (rng.normal(size=(4 * H, E)) * 0.02).astype(np.float32),
        "W_hh": (rng.normal(size=(4 * H, H)) * 0.02).astype(np.float32),
        "b_ih": (rng.normal(size=(4 * H,)) * 0.02).astype(np.float32),
        "b_hh": (rng.normal(size=(4 * H,)) * 0.02).astype(np.float32),
        "W_fc": (rng.normal(size=(V, H)) * 0.02).astype(np.float32),
        "b_fc": (rng.normal(size=(V,)) * 0.02).astype(np.float32),
    }
    o = kernel(**ins)
    print("out", o.shape, o.dtype, float(o[:, :31].sum()))

